# revision 40
# baseline (speedup 1.0000x reference)
"""Bidirectional GQA attention block (B=2, S=4096, D=768, 6 Q heads / 2 KV heads,
head_dim=128) on 8 Trainium2 NeuronCores.  ~260us HW exec (was 290us).

Sharding: core = b*4 + kvh*2 + sh
  b   in {0,1}: batch            (data parallel)
  kvh in {0,1}: kv-head group    (tensor parallel: 3 q-heads + 1 kv head each)
  sh  in {0,1}: query half       (sequence parallel on queries)
Each core computes K/V for its kv head over the full sequence, Q for its
2048-query chunk and 3 heads, unnormalized attention output transposed
(e x q), folds softmax normalization into a post-scale, and projects through
its 384 rows of wo.  Host sums the two kv-group partials per (b, sh).
y is returned in bf16 (tolerance is 2e-2; halves the output DMA).

Layout: all matmuls keep the contraction dim on partitions by feeding x
TRANSPOSED (host-side transpose, tiled per 128-partition block; each core's
own query half is fed first so one program serves all cores).  Scores are
computed transposed (S^T[ks, q]), exp'd without max subtraction (logits are
bounded ~ +-8 for randn inputs), and the AV matmul consumes P^T directly.
The softmax denominator is accumulated with one wide [128, 3, 512] bf16 add
per score group, folded at block end, partition-reduced+broadcast by a single
all-ones stationary matmul, and folded into the attn output as a fast
approximate reciprocal before the wo projection.

Schedule (the engines are balanced: PE matmul ~233us busy, ACT exp 198us,
DVE ~190us; runtime 260us):
  - ONE pool scope; x is DMA'd once per 512-seq chunk; projection chains
    (K/V/Q, 512-free matmuls, PSUM tag shared with attention) are pumped on
    demand between attention groups, so exp starts ~20us in.
  - V projection packs its four 128-row chains into a single PSUM bank
    (start=True clears only has_written bits, not data).
  - AV retires are deferred while projection chains remain (keeps both 'av'
    PSUM slots free so chain->cast pipelines), then catch up at 4/3 per step.
  - Scores lead AV by one group, two at block boundaries, so ACT never waits
    for the first exp of a block.
  - wo chains are queued and paced one per 3 attention slots, 2 slots after
    the tail that produced them; the final drain alternates PSUM tags so the
    chains do not serialize through two banks.
  - Block tails copy the AV PSUM to SBUF first (frees the bank), and the
    last block multiplies straight from PSUM.

Numerics: all matmul operands bf16 with fp32 PSUM accumulation; denominators
in bf16; rel err ~5.2e-3 vs fp32 reference (gate 2e-2).

Things that did NOT work on TRN2 (measured): any GpSimd compute (tensor_add
chains, partition_all_reduce) slows DVE/ACT 1.5-2x via SBUF contention and
its own ops run 3-5x slower than the cost model; scores lead of 2 groups
everywhere inflates all engines ~15%; upfront bursts of all 8 chunk DMAs
delay the critical first chunk; bf16 PSUM matmul output is TRN3-only.
"""

import numpy as np
import ml_dtypes

import concourse.bass as bass
import concourse.mybir as mybir
import concourse.tile as tile
from concourse import bacc
from concourse import bass_isa
from concourse.bass_utils import run_bass_kernel_spmd

# problem constants (hardcoded; harness supplies exactly these shapes)
B, S, D = 2, 4096, 768
N_HEADS, N_KV, HD = 6, 2, 128
GH = N_HEADS // N_KV          # q-heads per kv group = 3
QC = S // 2                   # per-core query chunk = 2048
P = 128                       # partitions
NB = D // P                   # 6 contraction blocks
ST = S // P                   # 32 key tiles
SC = 512                      # s-chunk for projections
NCH = S // SC                 # 8 chunks
QB = 512                      # q block in attention
GROUPS = [3] * 10 + [2]       # ks-tiles per score/exp group (sum = 32)
GT = 3                        # max group size
SCALE = 1.0 / float(np.sqrt(HD))

VPACK = True                  # pack V-proj chains into one PSUM bank
USE_GPR = False               # softmax-denominator partition reduce on GpSimd

FP32 = mybir.dt.float32
BF16 = mybir.dt.bfloat16
BF = ml_dtypes.bfloat16


def _emit(tc, xT, wq3, wk1, wv1, wo3, y):
    nc = tc.nc
    Exp = mybir.ActivationFunctionType.Exp

    with tc.tile_pool(name="persist", bufs=1) as persist:
        kT = persist.tile([P, NCH, SC], BF16)      # K^T [e, slot, ks]
        vS = persist.tile([P, NCH, 4, HD], BF16)   # V   [s%128, slot, t4, e]
        qT = persist.tile([P, GH, QC], BF16)       # Q^T [e, h, q]
        attT = persist.tile([P, GH, QC], BF16)     # normalized attn^T [e, h, q]
        wo_s = persist.tile([P, GH, D], BF16)
        ones_sq = persist.tile([P, P], BF16)
        nc.vector.memset(ones_sq, 1.0)

        with tc.tile_pool(name="p1w", bufs=1) as p1w, \
             tc.tile_pool(name="p1x", bufs=4) as p1x, \
             tc.tile_pool(name="p2ps", bufs=2, space="PSUM") as p2ps, \
             tc.tile_pool(name="p2av", bufs=2, space="PSUM") as p2av, \
             tc.tile_pool(name="p2p", bufs=6) as p2p, \
             tc.tile_pool(name="p2sb", bufs=3) as p2sb:
            # PE p-state warm-up: the HAM clock-gate ramps with activity;
            # burn dummy matmuls during the initial DMA wait (PE is idle
            # anyway) so the first real chains run at full clock.
            warm = p2av.tile([P, SC], FP32, tag="av", name="warm")
            for i in range(40):
                nc.tensor.matmul(warm[:, :P], lhsT=ones_sq, rhs=ones_sq,
                                 start=True, stop=True,
                                 skip_group_check=True)
            wq_s = p1w.tile([P, NB, GH * HD], BF16)
            wk_s = p1w.tile([P, NB, HD], BF16)
            wv_s = p1w.tile([P, NB, HD], BF16)

            xts = [p1x.tile([P, NB, SC], BF16, tag="xt", bufs=4,
                            name=f"xt_{j}") for j in range(NCH)]
            nc.sync.dma_start(out=wk_s, in_=wk1)
            # chunk 0 in 2-db pieces so the K chain's first matmuls start
            # before the whole chunk lands
            for db in range(0, NB, 2):
                nc.sync.dma_start(out=xts[0][:, db:db + 2, :],
                                  in_=xT[0][:, db:db + 2, :])
            nc.sync.dma_start(out=wv_s, in_=wv1)
            nc.sync.dma_start(out=wq_s, in_=wq3)
            nc.sync.dma_start(out=xts[1], in_=xT[1])
            nc.sync.dma_start(out=wo_s, in_=wo3)

            # ---- projection chains, emitted on demand between attention ----
            from collections import deque
            chains = deque()
            for c in range(NCH):
                chains.append(("K", c, 0))
                chains.append(("V", c, 0))
                if c < 4:
                    for h in range(GH):
                        chains.append(("Q", c, h))
            dma_next = [2]
            kdone = [False] * NCH
            vdone = [False] * NCH
            qdone = [[False] * GH for _ in range(4)]

            def emit_chain(part):
                kind, c, h = part
                if kind == "K" and dma_next[0] < NCH:
                    nc.sync.dma_start(out=xts[dma_next[0]],
                                      in_=xT[dma_next[0]])
                    dma_next[0] += 1
                xt = xts[c]
                if kind == "K":
                    kps = p2av.tile([P, SC], FP32, tag="av", name=f"kps_{c}")
                    for db in range(NB):
                        nc.tensor.matmul(kps, lhsT=wk_s[:, db, :],
                                         rhs=xt[:, db, :],
                                         start=db == 0, stop=db == NB - 1)
                    nc.vector.tensor_copy(kT[:, c, :], kps)
                    kdone[c] = True
                elif kind == "V":
                    # four 128-row chains packed into one PSUM bank
                    vps = p2av.tile([P, SC], FP32, tag="av", name=f"vps_{c}")
                    for t4 in range(4):
                        for db in range(NB):
                            nc.tensor.matmul(vps[:, t4 * HD:(t4 + 1) * HD],
                                             lhsT=xt[:, db, t4 * P:(t4 + 1) * P],
                                             rhs=wv_s[:, db, :],
                                             start=db == 0, stop=db == NB - 1,
                                             skip_group_check=True)
                    nc.vector.tensor_copy(vS[:, c, :, :], vps)
                    vdone[c] = True
                else:
                    qps = p2av.tile([P, SC], FP32, tag="av",
                                    name=f"qps_{c}_{h}")
                    for db in range(NB):
                        nc.tensor.matmul(qps,
                                         lhsT=wq_s[:, db, h * HD:(h + 1) * HD],
                                         rhs=xt[:, db, :],
                                         start=db == 0, stop=db == NB - 1)
                    nc.vector.tensor_copy(qT[:, h, c * SC:(c + 1) * SC], qps)
                    qdone[c][h] = True

            def pump(pred):
                while chains and not pred():
                    emit_chain(chains.popleft())

            # -- attention (scores transposed, dense, no max-subtraction) --
            blocks = [(qb, h) for qb in range(QC // QB) for h in range(GH)]
            state = {}
            wo_pending = []

            def emit_scores_exp(bi, gi):
                qb, h = blocks[bi]
                qsl = slice(qb * QB, (qb + 1) * QB)
                gsz = GROUPS[gi]
                kst0 = sum(GROUPS[:gi])
                kreq = (kst0 + gsz - 1) // 4
                pump(lambda: kdone[kreq] and qdone[qb][h])
                sps = p2ps.tile([P, GT, QB], FP32, tag="sps",
                                name=f"sps_{bi}_{gi}")
                for t in range(gsz):
                    kst = kst0 + t
                    nc.tensor.matmul(
                        sps[:, t, :],
                        lhsT=kT[:, kst // 4, (kst % 4) * P:(kst % 4 + 1) * P],
                        rhs=qT[:, h, qsl],
                        start=True, stop=True)
                pT = p2p.tile([P, GT, QB], BF16, tag="pT", bufs=14,
                               name=f"pT_{bi}_{gi}")
                nc.scalar.activation(pT[:, :gsz, :], sps[:, :gsz, :],
                                     Exp, scale=SCALE)
                return pT

            def emit_av_adds(bi, gi, pT):
                st = state.setdefault(bi, {"avps": None, "acc": None,
                                           "accp": None})
                gsz = GROUPS[gi]
                kst0 = sum(GROUPS[:gi])
                pump(lambda: vdone[(kst0 + gsz - 1) // 4])
                if st["avps"] is None:
                    st["avps"] = p2av.tile([P, QB], FP32, tag="av",
                                           name=f"avps_{bi}")
                # denominator: one wide add per group (acc[e, t, q] += pT)
                accn = p2sb.tile([P, GT, QB], BF16, tag="dacc", bufs=3,
                                 name=f"dacc_{bi}_{gi}")
                if gi == 0:
                    nc.vector.tensor_copy(accn, pT)
                else:
                    nc.vector.tensor_add(accn[:, :gsz, :],
                                         st["acc"][:, :gsz, :],
                                         pT[:, :gsz, :])
                st["accp"] = st["acc"]
                st["acc"] = accn
                for t in range(gsz):
                    kst = kst0 + t
                    nc.tensor.matmul(st["avps"],
                                     lhsT=vS[:, kst // 4, kst % 4, :],
                                     rhs=pT[:, t, :],
                                     start=kst == 0, stop=kst == ST - 1)

            def emit_block_tail(bi):
                qb, h = blocks[bi]
                qsl = slice(qb * QB, (qb + 1) * QB)
                st = state.pop(bi)
                # free the AV PSUM bank first so the next block's AV chain
                # never waits on the normalization chain below.  (The last
                # block has no successor: read the PSUM directly.)
                if bi == len(blocks) - 1:
                    avcp = st["avps"]
                else:
                    avcp = p2sb.tile([P, QB], FP32, tag="avcp", bufs=3,
                                     name=f"avcp_{bi}")
                    nc.vector.tensor_copy(avcp, st["avps"])
                # fold the [P, 3, QB] accumulator; plane 2 stopped at the
                # previous group (the last group has only 2 ks-tiles).
                dAB = p2sb.tile([P, QB], BF16, tag="dAB", bufs=2,
                                name=f"dAB_{bi}")
                nc.vector.tensor_add(dAB, st["acc"][:, 0, :],
                                     st["acc"][:, 1, :])
                dsum = p2sb.tile([P, QB], BF16, tag="dsum", bufs=2,
                                 name=f"dsum_{bi}")
                nc.vector.tensor_add(dsum, dAB, st["accp"][:, 2, :])
                den_s = p2av.tile([P, QB], FP32, tag="av", name=f"den_{bi}")
                nc.tensor.matmul(den_s, lhsT=ones_sq, rhs=dsum,
                                 start=True, stop=True)
                rb = p2sb.tile([P, QB], FP32, tag="rb", bufs=3,
                               name=f"rb_{bi}")
                nc.vector.reciprocal_approx_fast(rb, den_s)
                nc.vector.tensor_mul(attT[:, h, qsl], avcp, rb)
                if h == GH - 1:
                    queue_wo(qb)

            def queue_wo(qb):
                for qt4 in range(QB // P):
                    qt = qb * (QB // P) + qt4
                    ysb = p2sb.tile([P, D], BF16, tag="ysb", bufs=4,
                                    name=f"ysb_{qt}")
                    for c0, cn in ((0, 512), (512, 256)):
                        wo_pending.append((qt, c0, cn, ysb))

            def emit_one_wo(drain=False):
                if not wo_pending:
                    return
                qt, c0, cn, ysb = wo_pending.pop(0)
                if drain and (qt + (c0 > 0)) % 2:
                    # final drain: alternate into the (now idle) scores tag so
                    # chains don't serialize through the two 'av' banks.
                    wide = p2ps.tile([P, GT, QB], FP32, tag="sps",
                                     name=f"ypsw_{qt}_{c0}")
                    yps = wide[:, 0, :]
                else:
                    yps = p2av.tile([P, 512], FP32, tag="av",
                                    name=f"yps_{qt}_{c0}")
                for eb in range(GH):
                    nc.tensor.matmul(yps[:, :cn],
                                     lhsT=attT[:, eb, qt * P:(qt + 1) * P],
                                     rhs=wo_s[:, eb, c0:c0 + cn],
                                     start=eb == 0, stop=eb == GH - 1)
                nc.vector.tensor_copy(ysb[:, c0:c0 + cn], yps[:, :cn])
                if c0 == 512:
                    nc.sync.dma_start(out=y[qt * P:(qt + 1) * P, :], in_=ysb)

            # scores lead AV by 1 group (2 at block boundaries so ACT never
            # waits there); wo chains are paced one per 4 slots, starting 2
            # slots after the tail that produced them.  Projection chains are
            # pulled in on demand by the pump() calls above, which interleaves
            # them naturally with early attention groups.
            NG = len(GROUPS)
            seq = [(bi, gi) for bi in range(len(blocks)) for gi in range(NG)]
            pTs = {}
            last_tail_k = [-100]

            def retire(k, at):
                bi, gi = seq[k]
                emit_av_adds(bi, gi, pTs.pop(k))
                if gi == NG - 1:
                    emit_block_tail(bi)
                    last_tail_k[0] = at

            E = 0
            R = 0                  # next group to retire
            for k, (bi, gi) in enumerate(seq):
                lead_to = k + (1 if gi == NG - 1 else 0)
                while E <= min(lead_to, len(seq) - 1):
                    pTs[E] = emit_scores_exp(*seq[E])
                    E += 1
                # While projection chains are still being pumped, defer AV
                # retires: allocating avps would pin one of the two 'av'
                # PSUM slots and serialize chain->cast->chain.  (The chunk
                # region is DMA-bound, so the deferred AV work costs
                # nothing later.)  Catch up at 2 retires per step after.
                budget = 0 if not kdone[NCH - 1] else (2 if k % 3 == 0 else 1)
                while budget > 0 and R <= k - 1:
                    retire(R, k)
                    R += 1
                    budget -= 1
                dk = k - last_tail_k[0]
                if dk >= 2 and (dk - 2) % 3 == 0:
                    emit_one_wo()
            while R < len(seq):
                retire(R, len(seq))
                R += 1
            while wo_pending:
                emit_one_wo(drain=True)


def _build_nc():
    nc = bacc.Bacc("TRN2", target_bir_lowering=False, debug=False, num_devices=8)
    xT = nc.dram_tensor("xT", [NCH, P, NB, SC], BF16, kind="ExternalInput").ap()
    wq3 = nc.dram_tensor("wq3", [P, NB, GH * HD], BF16, kind="ExternalInput").ap()
    wk1 = nc.dram_tensor("wk1", [P, NB, HD], BF16, kind="ExternalInput").ap()
    wv1 = nc.dram_tensor("wv1", [P, NB, HD], BF16, kind="ExternalInput").ap()
    wo3 = nc.dram_tensor("wo3", [P, GH, D], BF16, kind="ExternalInput").ap()
    y = nc.dram_tensor("y", [QC, D], BF16, kind="ExternalOutput").ap()
    with tile.TileContext(nc) as tc:
        _emit(tc, xT, wq3, wk1, wv1, wo3, y)
    nc.compile()
    return nc


_NC = None


def _get_nc():
    global _NC
    if _NC is None:
        _NC = _build_nc()
    return _NC


def make_in_maps(x, wq, wk, wv, wo):
    x = np.asarray(x, np.float32)
    in_maps = []
    for core in range(8):
        b, kvh, sh = core >> 2, (core >> 1) & 1, core & 1
        xTb = x[b].T.astype(BF)                      # [D, S]
        g0, g1 = kvh * GH * HD, (kvh + 1) * GH * HD

        def tile_dm(a):                              # [D, M] -> [P, NB, M]
            return np.ascontiguousarray(
                a.reshape(NB, P, a.shape[1]).transpose(1, 0, 2))

        def tile_x(a):                               # [D, S] -> [NCH, P, NB, SC]
            return a.reshape(NB, P, NCH, SC).transpose(2, 1, 0, 3)

        # our query half's chunks first (kernel assumes slots 0-3 are its
        # query chunks; key-side slot order is irrelevant)
        order = list(range(NCH)) if sh == 0 else \
            list(range(NCH // 2, NCH)) + list(range(NCH // 2))
        in_maps.append({
            "xT": np.ascontiguousarray(tile_x(xTb)[order]),
            "wq3": tile_dm(np.asarray(wq, np.float32)[:, g0:g1].astype(BF)),
            "wk1": tile_dm(np.asarray(wk, np.float32)[:, kvh * HD:(kvh + 1) * HD].astype(BF)),
            "wv1": tile_dm(np.asarray(wv, np.float32)[:, kvh * HD:(kvh + 1) * HD].astype(BF)),
            "wo3": np.ascontiguousarray(
                np.asarray(wo, np.float32)[g0:g1, :].astype(BF)
                .reshape(GH, P, D).transpose(1, 0, 2)),
        })
    return in_maps


def combine_outputs(results):
    """results: list of 8 per-core {name: array} dicts -> full [B, S, D] output."""
    y = np.zeros((B, S, D), np.float32)
    for b in range(B):
        for sh in range(2):
            c0 = b * 4 + 0 * 2 + sh
            c1 = b * 4 + 1 * 2 + sh
            y[b, sh * QC:(sh + 1) * QC, :] = (
                results[c0]["y"].astype(np.float32)
                + results[c1]["y"].astype(np.float32)
            )
    return y


def kernel(x, wq, wk, wv, wo, **run_kwargs):
    nc = _get_nc()
    in_maps = make_in_maps(x, wq, wk, wv, wo)
    res = run_bass_kernel_spmd(nc, in_maps, core_ids=list(range(8)), **run_kwargs)
    out = combine_outputs(res.results)
    if run_kwargs:
        kernel.last_result = res
    return out


if __name__ == "__main__":
    rng = np.random.default_rng(0)
    x = rng.standard_normal((B, S, D), dtype=np.float32)
    std = 1.0 / np.sqrt(D)
    wq = rng.standard_normal((D, N_HEADS * HD), dtype=np.float32) * std
    wk = rng.standard_normal((D, N_KV * HD), dtype=np.float32) * std
    wv = rng.standard_normal((D, N_KV * HD), dtype=np.float32) * std
    wo = rng.standard_normal((N_HEADS * HD, D), dtype=np.float32) * std
    y = kernel(x, wq, wk, wv, wo)
    print("kernel output", y.shape, y.dtype, float(np.abs(y).max()))


# revision 41
# speedup vs baseline: 1.0032x; 1.0032x over previous
"""Bidirectional GQA attention block (B=2, S=4096, D=768, 6 Q heads / 2 KV heads,
head_dim=128) on 8 Trainium2 NeuronCores.  ~260us HW exec (was 290us).

Sharding: core = b*4 + kvh*2 + sh
  b   in {0,1}: batch            (data parallel)
  kvh in {0,1}: kv-head group    (tensor parallel: 3 q-heads + 1 kv head each)
  sh  in {0,1}: query half       (sequence parallel on queries)
Each core computes K/V for its kv head over the full sequence, Q for its
2048-query chunk and 3 heads, unnormalized attention output transposed
(e x q), folds softmax normalization into a post-scale, and projects through
its 384 rows of wo.  Host sums the two kv-group partials per (b, sh).
y is returned in bf16 (tolerance is 2e-2; halves the output DMA).

Layout: all matmuls keep the contraction dim on partitions by feeding x
TRANSPOSED (host-side transpose, tiled per 128-partition block; each core's
own query half is fed first so one program serves all cores).  Scores are
computed transposed (S^T[ks, q]), exp'd without max subtraction (logits are
bounded ~ +-8 for randn inputs), and the AV matmul consumes P^T directly.
The softmax denominator is accumulated with one wide [128, 3, 512] bf16 add
per score group, folded at block end, partition-reduced+broadcast by a single
all-ones stationary matmul, and folded into the attn output as a fast
approximate reciprocal before the wo projection.

Schedule (the engines are balanced: PE matmul ~233us busy, ACT exp 198us,
DVE ~190us; runtime 260us):
  - ONE pool scope; x is DMA'd once per 512-seq chunk; projection chains
    (K/V/Q, 512-free matmuls, PSUM tag shared with attention) are pumped on
    demand between attention groups, so exp starts ~20us in.
  - V projection packs its four 128-row chains into a single PSUM bank
    (start=True clears only has_written bits, not data).
  - AV retires are deferred while projection chains remain (keeps both 'av'
    PSUM slots free so chain->cast pipelines), then catch up at 4/3 per step.
  - Scores lead AV by one group, two at block boundaries, so ACT never waits
    for the first exp of a block.
  - wo chains are queued and paced one per 3 attention slots, 2 slots after
    the tail that produced them; the final drain alternates PSUM tags so the
    chains do not serialize through two banks.
  - Block tails copy the AV PSUM to SBUF first (frees the bank), and the
    last block multiplies straight from PSUM.

Numerics: all matmul operands bf16 with fp32 PSUM accumulation; denominators
in bf16; rel err ~5.2e-3 vs fp32 reference (gate 2e-2).

Things that did NOT work on TRN2 (measured): any GpSimd compute (tensor_add
chains, partition_all_reduce) slows DVE/ACT 1.5-2x via SBUF contention and
its own ops run 3-5x slower than the cost model; scores lead of 2 groups
everywhere inflates all engines ~15%; upfront bursts of all 8 chunk DMAs
delay the critical first chunk; bf16 PSUM matmul output is TRN3-only.
"""

import numpy as np
import ml_dtypes

import concourse.bass as bass
import concourse.mybir as mybir
import concourse.tile as tile
from concourse import bacc
from concourse import bass_isa
from concourse.bass_utils import run_bass_kernel_spmd

# problem constants (hardcoded; harness supplies exactly these shapes)
B, S, D = 2, 4096, 768
N_HEADS, N_KV, HD = 6, 2, 128
GH = N_HEADS // N_KV          # q-heads per kv group = 3
QC = S // 2                   # per-core query chunk = 2048
P = 128                       # partitions
NB = D // P                   # 6 contraction blocks
ST = S // P                   # 32 key tiles
SC = 512                      # s-chunk for projections
NCH = S // SC                 # 8 chunks
QB = 512                      # q block in attention
GROUPS = [3] * 10 + [2]       # ks-tiles per score/exp group (sum = 32)
GT = 3                        # max group size
SCALE = 1.0 / float(np.sqrt(HD))

VPACK = True                  # pack V-proj chains into one PSUM bank
USE_GPR = False               # softmax-denominator partition reduce on GpSimd

FP32 = mybir.dt.float32
BF16 = mybir.dt.bfloat16
BF = ml_dtypes.bfloat16


def _emit(tc, xT, wq3, wk1, wv1, wo3, y):
    nc = tc.nc
    Exp = mybir.ActivationFunctionType.Exp

    with tc.tile_pool(name="persist", bufs=1) as persist:
        kT = persist.tile([P, NCH, SC], BF16)      # K^T [e, slot, ks]
        vS = persist.tile([P, NCH, 4, HD], BF16)   # V   [s%128, slot, t4, e]
        qT = persist.tile([P, GH, QC], BF16)       # Q^T [e, h, q]
        attT = persist.tile([P, GH, QC], BF16)     # normalized attn^T [e, h, q]
        wo_s = persist.tile([P, GH, D], BF16)
        ones_sq = persist.tile([P, P], BF16)
        nc.vector.memset(ones_sq, 1.0)

        with tc.tile_pool(name="p1w", bufs=1) as p1w, \
             tc.tile_pool(name="p1x", bufs=4) as p1x, \
             tc.tile_pool(name="p2ps", bufs=2, space="PSUM") as p2ps, \
             tc.tile_pool(name="p2av", bufs=2, space="PSUM") as p2av, \
             tc.tile_pool(name="p2p", bufs=6) as p2p, \
             tc.tile_pool(name="p2sb", bufs=3) as p2sb:
            # PE p-state warm-up: the HAM clock-gate ramps with activity;
            # burn dummy matmuls during the initial DMA wait (PE is idle
            # anyway) so the first real chains run at full clock.
            warm = p2av.tile([P, SC], FP32, tag="av", name="warm")
            for i in range(16):
                nc.tensor.matmul(warm[:, :P], lhsT=ones_sq, rhs=ones_sq,
                                 start=True, stop=True,
                                 skip_group_check=True)
            wq_s = p1w.tile([P, NB, GH * HD], BF16)
            wk_s = p1w.tile([P, NB, HD], BF16)
            wv_s = p1w.tile([P, NB, HD], BF16)

            xts = [p1x.tile([P, NB, SC], BF16, tag="xt", bufs=4,
                            name=f"xt_{j}") for j in range(NCH)]
            nc.sync.dma_start(out=wk_s, in_=wk1)
            # second warm-up batch gated on wk's arrival: keeps the clock
            # ramped through the x0 transfer window, handing off hot to K0
            for i in range(12):
                nc.tensor.matmul(warm[:, :P], lhsT=ones_sq,
                                 rhs=wk_s[:, 0, :],
                                 start=True, stop=True,
                                 skip_group_check=True)
            # chunk 0 in 2-db pieces so the K chain's first matmuls start
            # before the whole chunk lands
            for db in range(0, NB, 2):
                nc.sync.dma_start(out=xts[0][:, db:db + 2, :],
                                  in_=xT[0][:, db:db + 2, :])
            nc.sync.dma_start(out=wv_s, in_=wv1)
            nc.sync.dma_start(out=wq_s, in_=wq3)
            nc.sync.dma_start(out=xts[1], in_=xT[1])
            nc.sync.dma_start(out=wo_s, in_=wo3)

            # ---- projection chains, emitted on demand between attention ----
            from collections import deque
            chains = deque()
            for c in range(NCH):
                chains.append(("K", c, 0))
                chains.append(("V", c, 0))
                if c < 4:
                    for h in range(GH):
                        chains.append(("Q", c, h))
            dma_next = [2]
            kdone = [False] * NCH
            vdone = [False] * NCH
            qdone = [[False] * GH for _ in range(4)]

            def emit_chain(part):
                kind, c, h = part
                if kind == "K" and dma_next[0] < NCH:
                    nc.sync.dma_start(out=xts[dma_next[0]],
                                      in_=xT[dma_next[0]])
                    dma_next[0] += 1
                xt = xts[c]
                if kind == "K":
                    kps = p2av.tile([P, SC], FP32, tag="av", name=f"kps_{c}")
                    for db in range(NB):
                        nc.tensor.matmul(kps, lhsT=wk_s[:, db, :],
                                         rhs=xt[:, db, :],
                                         start=db == 0, stop=db == NB - 1)
                    nc.vector.tensor_copy(kT[:, c, :], kps)
                    kdone[c] = True
                elif kind == "V":
                    # four 128-row chains packed into one PSUM bank
                    vps = p2av.tile([P, SC], FP32, tag="av", name=f"vps_{c}")
                    for t4 in range(4):
                        for db in range(NB):
                            nc.tensor.matmul(vps[:, t4 * HD:(t4 + 1) * HD],
                                             lhsT=xt[:, db, t4 * P:(t4 + 1) * P],
                                             rhs=wv_s[:, db, :],
                                             start=db == 0, stop=db == NB - 1,
                                             skip_group_check=True)
                    nc.vector.tensor_copy(vS[:, c, :, :], vps)
                    vdone[c] = True
                else:
                    qps = p2av.tile([P, SC], FP32, tag="av",
                                    name=f"qps_{c}_{h}")
                    for db in range(NB):
                        nc.tensor.matmul(qps,
                                         lhsT=wq_s[:, db, h * HD:(h + 1) * HD],
                                         rhs=xt[:, db, :],
                                         start=db == 0, stop=db == NB - 1)
                    nc.vector.tensor_copy(qT[:, h, c * SC:(c + 1) * SC], qps)
                    qdone[c][h] = True

            def pump(pred):
                while chains and not pred():
                    emit_chain(chains.popleft())

            # -- attention (scores transposed, dense, no max-subtraction) --
            blocks = [(qb, h) for qb in range(QC // QB) for h in range(GH)]
            state = {}
            wo_pending = []

            def emit_scores_exp(bi, gi):
                qb, h = blocks[bi]
                qsl = slice(qb * QB, (qb + 1) * QB)
                gsz = GROUPS[gi]
                kst0 = sum(GROUPS[:gi])
                kreq = (kst0 + gsz - 1) // 4
                pump(lambda: kdone[kreq] and qdone[qb][h])
                sps = p2ps.tile([P, GT, QB], FP32, tag="sps",
                                name=f"sps_{bi}_{gi}")
                for t in range(gsz):
                    kst = kst0 + t
                    nc.tensor.matmul(
                        sps[:, t, :],
                        lhsT=kT[:, kst // 4, (kst % 4) * P:(kst % 4 + 1) * P],
                        rhs=qT[:, h, qsl],
                        start=True, stop=True)
                pT = p2p.tile([P, GT, QB], BF16, tag="pT", bufs=14,
                               name=f"pT_{bi}_{gi}")
                nc.scalar.activation(pT[:, :gsz, :], sps[:, :gsz, :],
                                     Exp, scale=SCALE)
                return pT

            def emit_av_adds(bi, gi, pT):
                st = state.setdefault(bi, {"avps": None, "acc": None,
                                           "accp": None})
                gsz = GROUPS[gi]
                kst0 = sum(GROUPS[:gi])
                pump(lambda: vdone[(kst0 + gsz - 1) // 4])
                if st["avps"] is None:
                    st["avps"] = p2av.tile([P, QB], FP32, tag="av",
                                           name=f"avps_{bi}")
                # denominator: one wide add per group (acc[e, t, q] += pT)
                accn = p2sb.tile([P, GT, QB], BF16, tag="dacc", bufs=3,
                                 name=f"dacc_{bi}_{gi}")
                if gi == 0:
                    nc.vector.tensor_copy(accn, pT)
                else:
                    nc.vector.tensor_add(accn[:, :gsz, :],
                                         st["acc"][:, :gsz, :],
                                         pT[:, :gsz, :])
                st["accp"] = st["acc"]
                st["acc"] = accn
                for t in range(gsz):
                    kst = kst0 + t
                    nc.tensor.matmul(st["avps"],
                                     lhsT=vS[:, kst // 4, kst % 4, :],
                                     rhs=pT[:, t, :],
                                     start=kst == 0, stop=kst == ST - 1)

            def emit_block_tail(bi):
                qb, h = blocks[bi]
                qsl = slice(qb * QB, (qb + 1) * QB)
                st = state.pop(bi)
                # free the AV PSUM bank first so the next block's AV chain
                # never waits on the normalization chain below.  (The last
                # block has no successor: read the PSUM directly.)
                if bi == len(blocks) - 1:
                    avcp = st["avps"]
                else:
                    avcp = p2sb.tile([P, QB], FP32, tag="avcp", bufs=3,
                                     name=f"avcp_{bi}")
                    nc.vector.tensor_copy(avcp, st["avps"])
                # fold the [P, 3, QB] accumulator; plane 2 stopped at the
                # previous group (the last group has only 2 ks-tiles).
                dAB = p2sb.tile([P, QB], BF16, tag="dAB", bufs=2,
                                name=f"dAB_{bi}")
                nc.vector.tensor_add(dAB, st["acc"][:, 0, :],
                                     st["acc"][:, 1, :])
                dsum = p2sb.tile([P, QB], BF16, tag="dsum", bufs=2,
                                 name=f"dsum_{bi}")
                nc.vector.tensor_add(dsum, dAB, st["accp"][:, 2, :])
                den_s = p2av.tile([P, QB], FP32, tag="av", name=f"den_{bi}")
                nc.tensor.matmul(den_s, lhsT=ones_sq, rhs=dsum,
                                 start=True, stop=True)
                rb = p2sb.tile([P, QB], FP32, tag="rb", bufs=3,
                               name=f"rb_{bi}")
                nc.vector.reciprocal_approx_fast(rb, den_s)
                nc.vector.tensor_mul(attT[:, h, qsl], avcp, rb)
                if h == GH - 1:
                    queue_wo(qb)

            def queue_wo(qb):
                for qt4 in range(QB // P):
                    qt = qb * (QB // P) + qt4
                    ysb = p2sb.tile([P, D], BF16, tag="ysb", bufs=4,
                                    name=f"ysb_{qt}")
                    for c0, cn in ((0, 512), (512, 256)):
                        wo_pending.append((qt, c0, cn, ysb))

            def emit_one_wo(drain=False):
                if not wo_pending:
                    return
                qt, c0, cn, ysb = wo_pending.pop(0)
                if drain and (qt + (c0 > 0)) % 2:
                    # final drain: alternate into the (now idle) scores tag so
                    # chains don't serialize through the two 'av' banks.
                    wide = p2ps.tile([P, GT, QB], FP32, tag="sps",
                                     name=f"ypsw_{qt}_{c0}")
                    yps = wide[:, 0, :]
                else:
                    yps = p2av.tile([P, 512], FP32, tag="av",
                                    name=f"yps_{qt}_{c0}")
                for eb in range(GH):
                    nc.tensor.matmul(yps[:, :cn],
                                     lhsT=attT[:, eb, qt * P:(qt + 1) * P],
                                     rhs=wo_s[:, eb, c0:c0 + cn],
                                     start=eb == 0, stop=eb == GH - 1)
                nc.vector.tensor_copy(ysb[:, c0:c0 + cn], yps[:, :cn])
                if c0 == 512:
                    nc.sync.dma_start(out=y[qt * P:(qt + 1) * P, :], in_=ysb)

            # scores lead AV by 1 group (2 at block boundaries so ACT never
            # waits there); wo chains are paced one per 4 slots, starting 2
            # slots after the tail that produced them.  Projection chains are
            # pulled in on demand by the pump() calls above, which interleaves
            # them naturally with early attention groups.
            NG = len(GROUPS)
            seq = [(bi, gi) for bi in range(len(blocks)) for gi in range(NG)]
            pTs = {}
            last_tail_k = [-100]

            def retire(k, at):
                bi, gi = seq[k]
                emit_av_adds(bi, gi, pTs.pop(k))
                if gi == NG - 1:
                    emit_block_tail(bi)
                    last_tail_k[0] = at

            E = 0
            R = 0                  # next group to retire
            for k, (bi, gi) in enumerate(seq):
                lead_to = k + (1 if gi == NG - 1 else 0)
                while E <= min(lead_to, len(seq) - 1):
                    pTs[E] = emit_scores_exp(*seq[E])
                    E += 1
                # While projection chains are still being pumped, defer AV
                # retires: allocating avps would pin one of the two 'av'
                # PSUM slots and serialize chain->cast->chain.  (The chunk
                # region is DMA-bound, so the deferred AV work costs
                # nothing later.)  Catch up at 2 retires per step after.
                budget = 0 if not kdone[NCH - 1] else (2 if k % 3 == 0 else 1)
                while budget > 0 and R <= k - 1:
                    retire(R, k)
                    R += 1
                    budget -= 1
                dk = k - last_tail_k[0]
                if dk >= 2 and (dk - 2) % 3 == 0:
                    emit_one_wo()
            while R < len(seq):
                retire(R, len(seq))
                R += 1
            while wo_pending:
                emit_one_wo(drain=True)


def _build_nc():
    nc = bacc.Bacc("TRN2", target_bir_lowering=False, debug=False, num_devices=8)
    xT = nc.dram_tensor("xT", [NCH, P, NB, SC], BF16, kind="ExternalInput").ap()
    wq3 = nc.dram_tensor("wq3", [P, NB, GH * HD], BF16, kind="ExternalInput").ap()
    wk1 = nc.dram_tensor("wk1", [P, NB, HD], BF16, kind="ExternalInput").ap()
    wv1 = nc.dram_tensor("wv1", [P, NB, HD], BF16, kind="ExternalInput").ap()
    wo3 = nc.dram_tensor("wo3", [P, GH, D], BF16, kind="ExternalInput").ap()
    y = nc.dram_tensor("y", [QC, D], BF16, kind="ExternalOutput").ap()
    with tile.TileContext(nc) as tc:
        _emit(tc, xT, wq3, wk1, wv1, wo3, y)
    nc.compile()
    return nc


_NC = None


def _get_nc():
    global _NC
    if _NC is None:
        _NC = _build_nc()
    return _NC


def make_in_maps(x, wq, wk, wv, wo):
    x = np.asarray(x, np.float32)
    in_maps = []
    for core in range(8):
        b, kvh, sh = core >> 2, (core >> 1) & 1, core & 1
        xTb = x[b].T.astype(BF)                      # [D, S]
        g0, g1 = kvh * GH * HD, (kvh + 1) * GH * HD

        def tile_dm(a):                              # [D, M] -> [P, NB, M]
            return np.ascontiguousarray(
                a.reshape(NB, P, a.shape[1]).transpose(1, 0, 2))

        def tile_x(a):                               # [D, S] -> [NCH, P, NB, SC]
            return a.reshape(NB, P, NCH, SC).transpose(2, 1, 0, 3)

        # our query half's chunks first (kernel assumes slots 0-3 are its
        # query chunks; key-side slot order is irrelevant)
        order = list(range(NCH)) if sh == 0 else \
            list(range(NCH // 2, NCH)) + list(range(NCH // 2))
        in_maps.append({
            "xT": np.ascontiguousarray(tile_x(xTb)[order]),
            "wq3": tile_dm(np.asarray(wq, np.float32)[:, g0:g1].astype(BF)),
            "wk1": tile_dm(np.asarray(wk, np.float32)[:, kvh * HD:(kvh + 1) * HD].astype(BF)),
            "wv1": tile_dm(np.asarray(wv, np.float32)[:, kvh * HD:(kvh + 1) * HD].astype(BF)),
            "wo3": np.ascontiguousarray(
                np.asarray(wo, np.float32)[g0:g1, :].astype(BF)
                .reshape(GH, P, D).transpose(1, 0, 2)),
        })
    return in_maps


def combine_outputs(results):
    """results: list of 8 per-core {name: array} dicts -> full [B, S, D] output."""
    y = np.zeros((B, S, D), np.float32)
    for b in range(B):
        for sh in range(2):
            c0 = b * 4 + 0 * 2 + sh
            c1 = b * 4 + 1 * 2 + sh
            y[b, sh * QC:(sh + 1) * QC, :] = (
                results[c0]["y"].astype(np.float32)
                + results[c1]["y"].astype(np.float32)
            )
    return y


def kernel(x, wq, wk, wv, wo, **run_kwargs):
    nc = _get_nc()
    in_maps = make_in_maps(x, wq, wk, wv, wo)
    res = run_bass_kernel_spmd(nc, in_maps, core_ids=list(range(8)), **run_kwargs)
    out = combine_outputs(res.results)
    if run_kwargs:
        kernel.last_result = res
    return out


if __name__ == "__main__":
    rng = np.random.default_rng(0)
    x = rng.standard_normal((B, S, D), dtype=np.float32)
    std = 1.0 / np.sqrt(D)
    wq = rng.standard_normal((D, N_HEADS * HD), dtype=np.float32) * std
    wk = rng.standard_normal((D, N_KV * HD), dtype=np.float32) * std
    wv = rng.standard_normal((D, N_KV * HD), dtype=np.float32) * std
    wo = rng.standard_normal((N_HEADS * HD, D), dtype=np.float32) * std
    y = kernel(x, wq, wk, wv, wo)
    print("kernel output", y.shape, y.dtype, float(np.abs(y).max()))
